# revision 43
# baseline (speedup 1.0000x reference)
"""Trainium2 Bass kernel for DescriptorMatcher (mutual nearest neighbor matching).

Problem: given desc0 [B,N,D], desc1 [B,M,D] (B=4, N=M=8192, D=128, fp32):
    sim     = desc0 @ desc1^T                      [B,N,M]
    score0  = max_m sim                            [B,N]
    match01 = argmax_m sim                         [B,N]
    match10 = argmax_n sim                         [B,M]
    valid   = (match10[match01[n]] == n) & (score0 > 0.1)
returns (match01, score0, valid).

Key reformulations:
  1. mutual check needs no match10 indices:
       match10[match01[n]] == n  <=>  score0[n] == colmax[match01[n]]
  2. the whole pipeline runs on an fp16 value lattice: inputs are rounded to
     fp16, matmuls accumulate fp32 in PSUM, and every sim element is rounded
     to fp16 before any max. All maxes (row-chunk, column, cross-core) are
     exact on fp16 values, so the equality check is exact. Measured accuracy
     vs the fp32 reference: ~56/32768 match flips, well under the 2e-2 gate.
     fp16(max(a,b)) == max(fp16(a),fp16(b)) (monotone rounding), so folds may
     read fp32 PSUM directly and emit fp16.

Sharding: 8 cores = 4 batches x 2 row-halves (4096 rows each).

Phase 1 (per core): for each of 32 n-tiles [128 rows x 8192 cols]:
    PE  : 16 fp16 matmuls -> 4 PSUM tiles [128,2048] fp32 (1 cyc/row)
    ACT : cast cols [1024:8192] PSUM -> SBUF fp16  (7/8 pass)
    DVE : colaccD = max(colaccD, s16[colD cols])   (fp16 2x mode)
          fold tree X-zone chunks -> 64-wide floors (fp16 2x)
          merged floor reduce [128,16,FW] -> cm[128,16] per tile
    Pool: Z-zone (cols 0:1024) straight from PSUM: colaccZ max + fold tree
          W-zone fold tree (fp16), colaccP max
  colmax partition/cross-core reduction happens on HOST (colacc DMA'd out).
  Host: score0 = cm.max, cstar = cm.argmax (16 chunks of 512), group rows.

Phase 2 (per core): recompute sim[:, cstar-chunk] with identically laid-out
  fp16 matmuls + identical ACT cast (bit-exact vs phase 1), then
  max_index(score, chunk512) -> exact first-occurrence argmax position.
"""

import numpy as np

import concourse.bass as bass  # noqa: F401  (bass must import before tile)
import concourse.mybir as mybir
import concourse.tile as tile
from concourse import bacc

B, N, M, D = 4, 8192, 8192, 128
NCORES = 8
HALF = N // 2          # rows per core
NT = HALF // 128       # 32 n-tiles per core
CW = 512               # chunk width (argmax groups)
NCH = M // CW          # 16 chunks
PAD = 384              # phase-2 rows per chunk-group (mean 256, sigma ~15.5)
NST = NCH * PAD // 128  # 48 phase-2 sub-tiles

# phase-1 layout: ACT casts all 8 PSUM groups to fp16; DVE does the column
# accumulator and all fold trees (GPSIMD cannot run tensor ops on TRN2, and
# it cannot touch PSUM, so Pool is unusable here).
FW = 16                # fold-tree floor width (host does the final 16->1 max)


def _build1():
    f16, f32 = mybir.dt.float16, mybir.dt.float32
    mx = mybir.AluOpType.max
    nc = bacc.Bacc("TRN2", target_bir_lowering=False, debug=False,
                   num_devices=NCORES)
    at = nc.dram_tensor("at", [D, HALF], f16, kind="ExternalInput").ap()
    bt = nc.dram_tensor("bt", [D, M], f16, kind="ExternalInput").ap()
    cm_o = nc.dram_tensor("cm", [128, NT * NCH * FW], f16,
                          kind="ExternalOutput").ap()
    cd_o = nc.dram_tensor("cd", [128, M], f16, kind="ExternalOutput").ap()

    with tile.TileContext(nc) as tc:
        with tc.tile_pool(name="big", bufs=1) as big, \
             tc.tile_pool(name="rows", bufs=4) as rows, \
             tc.tile_pool(name="scr", bufs=3) as scr, \
             tc.tile_pool(name="ps", bufs=4, space="PSUM") as ps:
            atb = big.tile([128, HALF], f16, name="atb")
            btb = big.tile([128, M], f16, name="btb")
            # interleave so the first n-tile's operands land early
            nc.sync.dma_start(atb[:, 0:1024], at[:, 0:1024])
            for c in range(0, M, 1024):
                nc.sync.dma_start(btb[:, c:c + 1024], bt[:, c:c + 1024])
            for c in range(1024, HALF, 1024):
                nc.sync.dma_start(atb[:, c:c + 1024], at[:, c:c + 1024])
            colD = big.tile([128, M], f16, name="colD")

            def fold_tree(eng, src3d, nchunk, f_lo, F, tag):
                """Fold src3d [128, nchunk, CW] by halves down to FW, last
                level written into F[:, f_lo:f_lo+nchunk, :]."""
                cur, w = src3d, CW // 2
                while w >= FW:
                    if w == FW:
                        out = F[:, f_lo:f_lo + nchunk, :]
                    else:
                        out = scr.tile([128, nchunk, w], f16,
                                       tag=f"{tag}{w}", name=f"{tag}{w}")[:]
                    eng.tensor_tensor(out, cur[:, :, :w], cur[:, :, w:], op=mx)
                    cur, w = out, w // 2

            for t in range(NT):
                s16 = rows.tile([128, M], f16, tag="s16", name="s16")
                F = scr.tile([128, NCH, FW], f16, tag="F", name="F")
                for c in range(8):
                    pt = ps.tile([128, 1024], f32, tag="pt", name="pt")
                    for j in range(2):
                        mlo = c * 1024 + j * 512
                        nc.tensor.matmul(pt[:, j * 512:(j + 1) * 512],
                                         atb[:, t * 128:(t + 1) * 128],
                                         btb[:, mlo:mlo + 512],
                                         start=True, stop=True)
                    nc.scalar.copy(s16[:, c * 1024:(c + 1) * 1024], pt[:])

                # ---- DVE: fold tree over all 16 chunks (fp16)
                fold_tree(nc.vector,
                          s16[:].rearrange("p (c w) -> p c w", w=CW),
                          NCH, 0, F, "x")

                # ---- DVE: colD (full width, fp16 2x)
                if t == 0:
                    nc.vector.tensor_copy(colD[:], s16[:])
                else:
                    nc.vector.tensor_tensor(colD[:], colD[:], s16[:], op=mx)

                # stream this tile's floors out; host finishes the 16->1 max
                W = NCH * FW
                nc.sync.dma_start(cm_o[:, t * W:(t + 1) * W],
                                  F[:].rearrange("p c w -> p (c w)"))

            for c in range(0, M, 2048):
                nc.sync.dma_start(cd_o[:, c:c + 2048], colD[:, c:c + 2048])
    nc.compile()
    return nc


def _build2():
    f16, f32, u32 = mybir.dt.float16, mybir.dt.float32, mybir.dt.uint32
    nc = bacc.Bacc("TRN2", target_bir_lowering=False, debug=False,
                   num_devices=NCORES)
    at2 = nc.dram_tensor("at2", [D, NCH * PAD], f16, kind="ExternalInput").ap()
    bt = nc.dram_tensor("bt", [D, M], f16, kind="ExternalInput").ap()
    sg = nc.dram_tensor("sg", [128, NST], f16, kind="ExternalInput").ap()
    idx_o = nc.dram_tensor("idx", [128, NST * 8], u32, kind="ExternalOutput").ap()
    with tile.TileContext(nc) as tc:
        with tc.tile_pool(name="big", bufs=1) as big, \
             tc.tile_pool(name="work", bufs=6) as work, \
             tc.tile_pool(name="ps", bufs=4, space="PSUM") as ps:
            a2b = big.tile([128, NCH * PAD], f16, name="a2b")
            btb = big.tile([128, M], f16, name="btb")
            sgb = big.tile([128, NST], f16, name="sgb")
            # order matters: group 0's operands first, few serial issues
            nc.sync.dma_start(btb[:, 0:1024], bt[:, 0:1024])
            nc.sync.dma_start(a2b[:, 0:2 * PAD], at2[:, 0:2 * PAD])
            nc.sync.dma_start(sgb[:], sg[:])
            nc.sync.dma_start(a2b[:, 2 * PAD:8 * PAD], at2[:, 2 * PAD:8 * PAD])
            nc.sync.dma_start(btb[:, 1024:3072], bt[:, 1024:3072])
            nc.sync.dma_start(a2b[:, 8 * PAD:], at2[:, 8 * PAD:])
            for c in range(3072, M, 2048):
                hi = min(c + 2048, M)
                nc.sync.dma_start(btb[:, c:hi], bt[:, c:hi])
            idx8 = big.tile([128, NST * 8], u32, name="idx8")
            # broadcast all subtile scores to 8 copies each, once
            scb = big.tile([128, NST, 8], f16, name="scb")
            nc.vector.tensor_copy(
                scb[:], sgb[:].rearrange("p (c o) -> p c o", o=1)
                .to_broadcast((128, NST, 8)))
            KP = PAD // 128
            for p in range(NST // 2):
                st0 = 2 * p
                pt = ps.tile([128, 2 * CW], f32, tag="pt", name="pt")
                for k in range(2):
                    st = st0 + k
                    g = st // KP
                    nc.tensor.matmul(pt[:, k * CW:(k + 1) * CW],
                                     a2b[:, st * 128:(st + 1) * 128],
                                     btb[:, g * CW:(g + 1) * CW],
                                     start=True, stop=True)
                ch = work.tile([128, 2 * CW], f16, tag="ch", name="ch")
                nc.scalar.copy(ch[:], pt[:])
                for k in range(2):
                    st = st0 + k
                    nc.vector.max_index(idx8[:, st * 8:(st + 1) * 8],
                                        scb[:, st, :],
                                        ch[:, k * CW:(k + 1) * CW])
            nc.sync.dma_start(idx_o[:], idx8[:])
    nc.compile()
    return nc


_cached = None


def _make_exec(nc):
    import jax
    from jax.sharding import Mesh, PartitionSpec
    from jax.experimental.shard_map import shard_map
    from concourse import bass2jax
    from concourse.bass2jax import _bass_exec_p

    partition_name = nc.partition_id_tensor.name if nc.partition_id_tensor else None
    in_names, out_names, out_avals, out_shapes = [], [], [], []
    for alloc in nc.m.functions[0].allocations:
        if not isinstance(alloc, mybir.MemoryLocationSet):
            continue
        name = alloc.memorylocations[0].name
        if alloc.kind == "ExternalInput":
            if name != partition_name:
                in_names.append(name)
        elif alloc.kind == "ExternalOutput":
            shape = tuple(alloc.tensor_shape)
            dtype = mybir.dt.np(alloc.dtype)
            out_names.append(name)
            out_shapes.append((shape, dtype))
            out_avals.append(jax.core.ShapedArray(shape, dtype))
    n_params = len(in_names)
    n_outs = len(out_names)
    all_in_names = in_names + out_names
    if partition_name is not None:
        all_in_names = all_in_names + [partition_name]

    def _body(*args):
        operands = list(args)
        if partition_name is not None:
            operands.append(bass2jax.partition_id_tensor())
        outs = _bass_exec_p.bind(
            *operands, out_avals=tuple(out_avals), in_names=tuple(all_in_names),
            out_names=tuple(out_names), lowering_input_output_aliases=(),
            sim_require_finite=True, sim_require_nnan=True, nc=nc)
        return tuple(outs)

    devices = jax.devices()[:NCORES]
    mesh = Mesh(np.asarray(devices), ("core",))
    in_specs = (PartitionSpec("core"),) * (n_params + n_outs)
    out_specs = (PartitionSpec("core"),) * n_outs
    fn = jax.jit(shard_map(_body, mesh=mesh, in_specs=in_specs,
                           out_specs=out_specs, check_rep=False),
                 keep_unused=True)
    return {"fn": fn, "in_names": in_names, "out_names": out_names,
            "out_shapes": out_shapes, "nc": nc}


def _run(ex, ins):
    """ins: dict name -> [NCORES, *shape]; returns dict name -> [NCORES, *shape]."""
    concat_in = [np.ascontiguousarray(ins[n].reshape(-1, *ins[n].shape[2:]))
                 for n in ex["in_names"]]
    concat_zeros = [np.zeros((NCORES * s[0], *s[1:]), dt)
                    for (s, dt) in ex["out_shapes"]]
    out_arrs = ex["fn"](*concat_in, *concat_zeros)
    return {name: np.asarray(out_arrs[i]).reshape(NCORES, *ex["out_shapes"][i][0])
            for i, name in enumerate(ex["out_names"])}


def kernel(desc0, desc1):
    global _cached
    desc0 = np.asarray(desc0, dtype=np.float32)
    desc1 = np.asarray(desc1, dtype=np.float32)
    assert desc0.shape == (B, N, D) and desc1.shape == (B, M, D)

    if _cached is None:
        _cached = (_make_exec(_build1()), _make_exec(_build2()))
    ex1, ex2 = _cached

    a16 = desc0.astype(np.float16)
    b16 = desc1.astype(np.float16)
    a_slab = np.stack([a16[b, h * HALF:(h + 1) * HALF]
                       for b in range(B) for h in range(2)])      # [8,4096,128]
    bt_all = np.stack([b16[b].transpose(1, 0)
                       for b in range(B) for h in range(2)])      # [8,128,8192]
    at_all = np.ascontiguousarray(a_slab.transpose(0, 2, 1))      # [8,128,4096]

    r1 = _run(ex1, {"at": at_all, "bt": bt_all})

    # host glue: finish floor max, colmax assembly, chunk-argmax, grouping
    cm = r1["cm"].reshape(NCORES, 128, NT, NCH, FW).max(axis=4) \
                 .transpose(0, 2, 1, 3).reshape(NCORES, HALF, NCH)
    score0_c = cm.max(axis=2)                                     # [8,4096] f16
    cstar_c = cm.argmax(axis=2)                                   # [8,4096]

    colmax = r1["cd"].reshape(B, 2 * 128, M).max(axis=1)          # [B,M] f16

    at2 = np.zeros((NCORES, D, NCH * PAD), np.float16)
    sgv = np.full((NCORES, 128, NST), np.inf, np.float16)
    slot_of_row = np.full((NCORES, HALF), -1, np.int64)
    overflow = []                                                 # (core, row)
    for core in range(NCORES):
        for g in range(NCH):
            rows = np.nonzero(cstar_c[core] == g)[0]
            if len(rows) > PAD:
                overflow.extend((core, r) for r in rows[PAD:])
                rows = rows[:PAD]
            slots = g * PAD + np.arange(len(rows))
            slot_of_row[core, rows] = slots
            at2[core][:, slots] = a_slab[core][rows].T
            sgv[core][slots % 128, slots // 128] = score0_c[core][rows]

    r2 = _run(ex2, {"at2": at2, "bt": bt_all, "sg": sgv})
    within = r2["idx"][:, :, ::8]                                 # [8,128,NST]

    match01 = np.empty((B, N), dtype=np.int32)
    score0 = np.empty((B, N), dtype=np.float32)
    valid = np.empty((B, N), dtype=bool)

    for core in range(NCORES):
        b, h = divmod(core, 2)
        s = score0_c[core].astype(np.float32)
        sl = slot_of_row[core]
        m = cstar_c[core] * CW + \
            within[core][sl % 128, sl // 128].astype(np.int64)
        sel = slice(h * HALF, (h + 1) * HALF)
        score0[b, sel] = s
        match01[b, sel] = m.astype(np.int32)
        valid[b, sel] = (s > 0.1) & (score0_c[core] == colmax[b][m])

    for core, row in overflow:                                    # ~never taken
        b, h = divmod(core, 2)
        simrow = (a_slab[core][row].astype(np.float32)
                  @ b16[b].astype(np.float32).T).astype(np.float16)
        n = h * HALF + row
        match01[b, n] = int(simrow.argmax())
        score0[b, n] = float(simrow.max())
        valid[b, n] = (score0[b, n] > 0.1) & \
                      (simrow.max() == colmax[b][match01[b, n]])

    return match01, score0, valid


# revision 45
# speedup vs baseline: 1.0429x; 1.0429x over previous
"""Trainium2 Bass kernel for DescriptorMatcher (mutual nearest neighbor matching).

Problem: given desc0 [B,N,D], desc1 [B,M,D] (B=4, N=M=8192, D=128, fp32):
    sim     = desc0 @ desc1^T                      [B,N,M]
    score0  = max_m sim                            [B,N]
    match01 = argmax_m sim                         [B,N]
    match10 = argmax_n sim                         [B,M]
    valid   = (match10[match01[n]] == n) & (score0 > 0.1)
returns (match01, score0, valid).

Key reformulations:
  1. mutual check needs no match10 indices:
       match10[match01[n]] == n  <=>  score0[n] == colmax[match01[n]]
  2. the whole pipeline runs on an fp16 value lattice: inputs are rounded to
     fp16, matmuls accumulate fp32 in PSUM, and every sim element is rounded
     to fp16 before any max. All maxes (row-chunk, column, cross-core) are
     exact on fp16 values, so the equality check is exact. Measured accuracy
     vs the fp32 reference: ~56/32768 match flips, well under the 2e-2 gate.
     fp16(max(a,b)) == max(fp16(a),fp16(b)) (monotone rounding), so folds may
     read fp32 PSUM directly and emit fp16.

Sharding: 8 cores = 4 batches x 2 row-halves (4096 rows each).

Phase 1 (per core): for each of 32 n-tiles [128 rows x 8192 cols]:
    PE  : 16 fp16 matmuls -> 8 PSUM tiles [128,1024] fp32 (1 cyc/row)
    ACT : cast all 8 PSUM groups -> SBUF fp16 (GPSIMD can't run tensor ops
          or touch PSUM on this toolchain, so Pool is unusable)
    DVE : colD = max(colD, s16) full-width (fp16 2x mode)
          fold tree 16 chunks -> 16-wide floors (fp16 2x), streamed to HBM
  colmax partition/cross-core reduction and the floors' final 16->1 max
  happen on HOST. Host: score0, cstar = argmax over 16 chunks, group rows.

Phase 2 (per core): recompute sim[:, cstar-chunk] with identically laid-out
  fp16 matmuls + identical ACT cast (bit-exact vs phase 1), then
  max_index(score, chunk512) -> exact first-occurrence argmax position.
"""

import numpy as np

import concourse.bass as bass  # noqa: F401  (bass must import before tile)
import concourse.mybir as mybir
import concourse.tile as tile
from concourse import bacc

B, N, M, D = 4, 8192, 8192, 128
NCORES = 8
HALF = N // 2          # rows per core
NT = HALF // 128       # 32 n-tiles per core
CW = 512               # chunk width (argmax groups)
NCH = M // CW          # 16 chunks
PAD = 384              # phase-2 rows per chunk-group (mean 256, sigma ~15.5)
NST = NCH * PAD // 128  # 48 phase-2 sub-tiles

# phase-1 layout: ACT casts all 8 PSUM groups to fp16; DVE does the column
# accumulator and all fold trees (GPSIMD cannot run tensor ops on TRN2, and
# it cannot touch PSUM, so Pool is unusable here).
FW = 128               # fold-tree floor width (host does the final 128->1 max)


def _build1():
    f16, f32 = mybir.dt.float16, mybir.dt.float32
    mx = mybir.AluOpType.max
    nc = bacc.Bacc("TRN2", target_bir_lowering=False, debug=False,
                   num_devices=NCORES)
    at = nc.dram_tensor("at", [D, HALF], f16, kind="ExternalInput").ap()
    bt = nc.dram_tensor("bt", [D, M], f16, kind="ExternalInput").ap()
    cm_o = nc.dram_tensor("cm", [128, NT * NCH * FW], f16,
                          kind="ExternalOutput").ap()
    cd_o = nc.dram_tensor("cd", [128, M], f16, kind="ExternalOutput").ap()

    with tile.TileContext(nc) as tc:
        with tc.tile_pool(name="big", bufs=1) as big, \
             tc.tile_pool(name="rows", bufs=4) as rows, \
             tc.tile_pool(name="scr", bufs=3) as scr, \
             tc.tile_pool(name="ps", bufs=4, space="PSUM") as ps:
            atb = big.tile([128, HALF], f16, name="atb")
            btb = big.tile([128, M], f16, name="btb")
            # interleave so the first n-tile's operands land early
            nc.sync.dma_start(atb[:, 0:1024], at[:, 0:1024])
            for c in range(0, M, 1024):
                nc.sync.dma_start(btb[:, c:c + 1024], bt[:, c:c + 1024])
            for c in range(1024, HALF, 1024):
                nc.sync.dma_start(atb[:, c:c + 1024], at[:, c:c + 1024])
            colD = big.tile([128, M], f16, name="colD")

            def fold_tree(eng, src3d, nchunk, f_lo, F, tag):
                """Fold src3d [128, nchunk, CW] by halves down to FW, last
                level written into F[:, f_lo:f_lo+nchunk, :]."""
                cur, w = src3d, CW // 2
                while w >= FW:
                    if w == FW:
                        out = F[:, f_lo:f_lo + nchunk, :]
                    else:
                        out = scr.tile([128, nchunk, w], f16,
                                       tag=f"{tag}{w}", name=f"{tag}{w}")[:]
                    eng.tensor_tensor(out, cur[:, :, :w], cur[:, :, w:], op=mx)
                    cur, w = out, w // 2

            for t in range(NT):
                s16 = rows.tile([128, M], f16, tag="s16", name="s16")
                F = scr.tile([128, NCH, FW], f16, tag="F", name="F")
                for c in range(8):
                    pt = ps.tile([128, 1024], f32, tag="pt", name="pt")
                    for j in range(2):
                        mlo = c * 1024 + j * 512
                        nc.tensor.matmul(pt[:, j * 512:(j + 1) * 512],
                                         atb[:, t * 128:(t + 1) * 128],
                                         btb[:, mlo:mlo + 512],
                                         start=True, stop=True)
                    nc.scalar.copy(s16[:, c * 1024:(c + 1) * 1024], pt[:])

                # ---- DVE: fold tree over all 16 chunks (fp16)
                fold_tree(nc.vector,
                          s16[:].rearrange("p (c w) -> p c w", w=CW),
                          NCH, 0, F, "x")

                # ---- DVE: colD (full width, fp16 2x)
                if t == 0:
                    nc.vector.tensor_copy(colD[:], s16[:])
                else:
                    nc.vector.tensor_tensor(colD[:], colD[:], s16[:], op=mx)

                # stream this tile's floors out; host finishes the 16->1 max
                W = NCH * FW
                nc.sync.dma_start(cm_o[:, t * W:(t + 1) * W],
                                  F[:].rearrange("p c w -> p (c w)"))

            for c in range(0, M, 2048):
                nc.sync.dma_start(cd_o[:, c:c + 2048], colD[:, c:c + 2048])
    nc.compile()
    return nc


def _build2():
    f16, f32, u32 = mybir.dt.float16, mybir.dt.float32, mybir.dt.uint32
    nc = bacc.Bacc("TRN2", target_bir_lowering=False, debug=False,
                   num_devices=NCORES)
    at2 = nc.dram_tensor("at2", [D, NCH * PAD], f16, kind="ExternalInput").ap()
    bt = nc.dram_tensor("bt", [D, M], f16, kind="ExternalInput").ap()
    sg = nc.dram_tensor("sg", [128, NST], f16, kind="ExternalInput").ap()
    idx_o = nc.dram_tensor("idx", [128, NST * 8], u32, kind="ExternalOutput").ap()
    with tile.TileContext(nc) as tc:
        with tc.tile_pool(name="big", bufs=1) as big, \
             tc.tile_pool(name="work", bufs=6) as work, \
             tc.tile_pool(name="ps", bufs=4, space="PSUM") as ps:
            a2b = big.tile([128, NCH * PAD], f16, name="a2b")
            btb = big.tile([128, M], f16, name="btb")
            sgb = big.tile([128, NST], f16, name="sgb")
            # order matters: group 0's operands first, few serial issues
            nc.sync.dma_start(btb[:, 0:1024], bt[:, 0:1024])
            nc.sync.dma_start(a2b[:, 0:2 * PAD], at2[:, 0:2 * PAD])
            nc.sync.dma_start(sgb[:], sg[:])
            nc.sync.dma_start(a2b[:, 2 * PAD:8 * PAD], at2[:, 2 * PAD:8 * PAD])
            nc.sync.dma_start(btb[:, 1024:3072], bt[:, 1024:3072])
            nc.sync.dma_start(a2b[:, 8 * PAD:], at2[:, 8 * PAD:])
            for c in range(3072, M, 2048):
                hi = min(c + 2048, M)
                nc.sync.dma_start(btb[:, c:hi], bt[:, c:hi])
            idx8 = big.tile([128, NST * 8], u32, name="idx8")
            # broadcast all subtile scores to 8 copies each, once
            scb = big.tile([128, NST, 8], f16, name="scb")
            nc.vector.tensor_copy(
                scb[:], sgb[:].rearrange("p (c o) -> p c o", o=1)
                .to_broadcast((128, NST, 8)))
            KP = PAD // 128
            for p in range(NST // 2):
                st0 = 2 * p
                pt = ps.tile([128, 2 * CW], f32, tag="pt", name="pt")
                for k in range(2):
                    st = st0 + k
                    g = st // KP
                    nc.tensor.matmul(pt[:, k * CW:(k + 1) * CW],
                                     a2b[:, st * 128:(st + 1) * 128],
                                     btb[:, g * CW:(g + 1) * CW],
                                     start=True, stop=True)
                ch = work.tile([128, 2 * CW], f16, tag="ch", name="ch")
                nc.scalar.copy(ch[:], pt[:])
                for k in range(2):
                    st = st0 + k
                    nc.vector.max_index(idx8[:, st * 8:(st + 1) * 8],
                                        scb[:, st, :],
                                        ch[:, k * CW:(k + 1) * CW])
            nc.sync.dma_start(idx_o[:], idx8[:])
    nc.compile()
    return nc


_cached = None


def _make_exec(nc):
    import jax
    from jax.sharding import Mesh, PartitionSpec
    from jax.experimental.shard_map import shard_map
    from concourse import bass2jax
    from concourse.bass2jax import _bass_exec_p

    partition_name = nc.partition_id_tensor.name if nc.partition_id_tensor else None
    in_names, out_names, out_avals, out_shapes = [], [], [], []
    for alloc in nc.m.functions[0].allocations:
        if not isinstance(alloc, mybir.MemoryLocationSet):
            continue
        name = alloc.memorylocations[0].name
        if alloc.kind == "ExternalInput":
            if name != partition_name:
                in_names.append(name)
        elif alloc.kind == "ExternalOutput":
            shape = tuple(alloc.tensor_shape)
            dtype = mybir.dt.np(alloc.dtype)
            out_names.append(name)
            out_shapes.append((shape, dtype))
            out_avals.append(jax.core.ShapedArray(shape, dtype))
    n_params = len(in_names)
    n_outs = len(out_names)
    all_in_names = in_names + out_names
    if partition_name is not None:
        all_in_names = all_in_names + [partition_name]

    def _body(*args):
        operands = list(args)
        if partition_name is not None:
            operands.append(bass2jax.partition_id_tensor())
        outs = _bass_exec_p.bind(
            *operands, out_avals=tuple(out_avals), in_names=tuple(all_in_names),
            out_names=tuple(out_names), lowering_input_output_aliases=(),
            sim_require_finite=True, sim_require_nnan=True, nc=nc)
        return tuple(outs)

    devices = jax.devices()[:NCORES]
    mesh = Mesh(np.asarray(devices), ("core",))
    in_specs = (PartitionSpec("core"),) * (n_params + n_outs)
    out_specs = (PartitionSpec("core"),) * n_outs
    fn = jax.jit(shard_map(_body, mesh=mesh, in_specs=in_specs,
                           out_specs=out_specs, check_rep=False),
                 keep_unused=True)
    return {"fn": fn, "in_names": in_names, "out_names": out_names,
            "out_shapes": out_shapes, "nc": nc}


def _run(ex, ins):
    """ins: dict name -> [NCORES, *shape]; returns dict name -> [NCORES, *shape]."""
    concat_in = [np.ascontiguousarray(ins[n].reshape(-1, *ins[n].shape[2:]))
                 for n in ex["in_names"]]
    concat_zeros = [np.zeros((NCORES * s[0], *s[1:]), dt)
                    for (s, dt) in ex["out_shapes"]]
    out_arrs = ex["fn"](*concat_in, *concat_zeros)
    return {name: np.asarray(out_arrs[i]).reshape(NCORES, *ex["out_shapes"][i][0])
            for i, name in enumerate(ex["out_names"])}


def kernel(desc0, desc1):
    global _cached
    desc0 = np.asarray(desc0, dtype=np.float32)
    desc1 = np.asarray(desc1, dtype=np.float32)
    assert desc0.shape == (B, N, D) and desc1.shape == (B, M, D)

    if _cached is None:
        _cached = (_make_exec(_build1()), _make_exec(_build2()))
    ex1, ex2 = _cached

    a16 = desc0.astype(np.float16)
    b16 = desc1.astype(np.float16)
    a_slab = np.stack([a16[b, h * HALF:(h + 1) * HALF]
                       for b in range(B) for h in range(2)])      # [8,4096,128]
    bt_all = np.stack([b16[b].transpose(1, 0)
                       for b in range(B) for h in range(2)])      # [8,128,8192]
    at_all = np.ascontiguousarray(a_slab.transpose(0, 2, 1))      # [8,128,4096]

    r1 = _run(ex1, {"at": at_all, "bt": bt_all})

    # host glue: finish floor max, colmax assembly, chunk-argmax, grouping
    cm = r1["cm"].reshape(NCORES, 128, NT, NCH, FW).max(axis=4) \
                 .transpose(0, 2, 1, 3).reshape(NCORES, HALF, NCH)
    score0_c = cm.max(axis=2)                                     # [8,4096] f16
    cstar_c = cm.argmax(axis=2)                                   # [8,4096]

    colmax = r1["cd"].reshape(B, 2 * 128, M).max(axis=1)          # [B,M] f16

    at2 = np.zeros((NCORES, D, NCH * PAD), np.float16)
    sgv = np.full((NCORES, 128, NST), np.inf, np.float16)
    slot_of_row = np.full((NCORES, HALF), -1, np.int64)
    overflow = []                                                 # (core, row)
    for core in range(NCORES):
        for g in range(NCH):
            rows = np.nonzero(cstar_c[core] == g)[0]
            if len(rows) > PAD:
                overflow.extend((core, r) for r in rows[PAD:])
                rows = rows[:PAD]
            slots = g * PAD + np.arange(len(rows))
            slot_of_row[core, rows] = slots
            at2[core][:, slots] = a_slab[core][rows].T
            sgv[core][slots % 128, slots // 128] = score0_c[core][rows]

    r2 = _run(ex2, {"at2": at2, "bt": bt_all, "sg": sgv})
    within = r2["idx"][:, :, ::8]                                 # [8,128,NST]

    match01 = np.empty((B, N), dtype=np.int32)
    score0 = np.empty((B, N), dtype=np.float32)
    valid = np.empty((B, N), dtype=bool)

    for core in range(NCORES):
        b, h = divmod(core, 2)
        s = score0_c[core].astype(np.float32)
        sl = slot_of_row[core]
        m = cstar_c[core] * CW + \
            within[core][sl % 128, sl // 128].astype(np.int64)
        sel = slice(h * HALF, (h + 1) * HALF)
        score0[b, sel] = s
        match01[b, sel] = m.astype(np.int32)
        valid[b, sel] = (s > 0.1) & (score0_c[core] == colmax[b][m])

    for core, row in overflow:                                    # ~never taken
        b, h = divmod(core, 2)
        simrow = (a_slab[core][row].astype(np.float32)
                  @ b16[b].astype(np.float32).T).astype(np.float16)
        n = h * HALF + row
        match01[b, n] = int(simrow.argmax())
        score0[b, n] = float(simrow.max())
        valid[b, n] = (score0[b, n] > 0.1) & \
                      (simrow.max() == colmax[b][match01[b, n]])

    return match01, score0, valid


# revision 47
# speedup vs baseline: 1.1087x; 1.0632x over previous
"""Trainium2 Bass kernel for DescriptorMatcher (mutual nearest neighbor matching).

Problem: given desc0 [B,N,D], desc1 [B,M,D] (B=4, N=M=8192, D=128, fp32):
    sim     = desc0 @ desc1^T                      [B,N,M]
    score0  = max_m sim                            [B,N]
    match01 = argmax_m sim                         [B,N]
    match10 = argmax_n sim                         [B,M]
    valid   = (match10[match01[n]] == n) & (score0 > 0.1)
returns (match01, score0, valid).

Key reformulations:
  1. mutual check needs no match10 indices:
       match10[match01[n]] == n  <=>  score0[n] == colmax[match01[n]]
  2. the whole pipeline runs on an fp16 value lattice: inputs are rounded to
     fp16, matmuls accumulate fp32 in PSUM, and every sim element is rounded
     to fp16 before any max. All maxes (row-chunk, column, cross-core) are
     exact on fp16 values, so the equality check is exact. Measured accuracy
     vs the fp32 reference: ~56/32768 match flips, well under the 2e-2 gate.
     fp16(max(a,b)) == max(fp16(a),fp16(b)) (monotone rounding), so folds may
     read fp32 PSUM directly and emit fp16.

Sharding: 8 cores = 4 batches x 2 row-halves (4096 rows each).

Phase 1 (per core): for each of 32 n-tiles [128 rows x 8192 cols]:
    PE  : 16 fp16 matmuls -> 8 PSUM tiles [128,1024] fp32 (1 cyc/row)
    ACT : cast all 8 PSUM groups -> SBUF fp16 (GPSIMD can't run tensor ops
          or touch PSUM on this toolchain, so Pool is unusable)
    DVE : colD = max(colD, s16) full-width (fp16 2x mode)
          fold tree 16 chunks -> FW-wide floors (fp16 2x), streamed to HBM
  colmax partition/cross-core reduction and the floors' final FW->1 max
  happen on HOST. Host: score0, cstar = argmax over 16 chunks, group rows.

Phase 2 (per core): recompute sim[:, cstar-chunk] with identically laid-out
  fp16 matmuls + identical ACT cast (bit-exact vs phase 1), then
  max_index(score, chunk512) -> exact first-occurrence argmax position.
"""

import numpy as np

import concourse.bass as bass  # noqa: F401  (bass must import before tile)
import concourse.mybir as mybir
import concourse.tile as tile
from concourse import bacc

B, N, M, D = 4, 8192, 8192, 128
NCORES = 8
HALF = N // 2          # rows per core
NT = HALF // 128       # 32 n-tiles per core
CW = 512               # chunk width (argmax groups)
NCH = M // CW          # 16 chunks
PAD = 384              # phase-2 rows per chunk-group (mean 256, sigma ~15.5)
NST = NCH * PAD // 128  # 48 phase-2 sub-tiles

# phase-1 layout: ACT casts all 8 PSUM groups to fp16; DVE does the column
# accumulator and all fold trees (GPSIMD cannot run tensor ops on TRN2, and
# it cannot touch PSUM, so Pool is unusable here).
FW = 256               # fold-tree floor width (host finishes the 256->1 max)


def _build1():
    f16, f32 = mybir.dt.float16, mybir.dt.float32
    mx = mybir.AluOpType.max
    nc = bacc.Bacc("TRN2", target_bir_lowering=False, debug=False,
                   num_devices=NCORES)
    at = nc.dram_tensor("at", [D, HALF], f16, kind="ExternalInput").ap()
    bt = nc.dram_tensor("bt", [D, M], f16, kind="ExternalInput").ap()
    cm_o = nc.dram_tensor("cm", [128, NT * NCH * FW], f16,
                          kind="ExternalOutput").ap()
    cd_o = nc.dram_tensor("cd", [128, M], f16, kind="ExternalOutput").ap()

    with tile.TileContext(nc) as tc:
        with tc.tile_pool(name="big", bufs=1) as big, \
             tc.tile_pool(name="rows", bufs=4) as rows, \
             tc.tile_pool(name="scr", bufs=3) as scr, \
             tc.tile_pool(name="ps", bufs=4, space="PSUM") as ps:
            atb = big.tile([128, HALF], f16, name="atb")
            btb = big.tile([128, M], f16, name="btb")
            # interleave so the first n-tile's operands land early
            nc.sync.dma_start(atb[:, 0:1024], at[:, 0:1024])
            for c in range(0, M, 1024):
                nc.sync.dma_start(btb[:, c:c + 1024], bt[:, c:c + 1024])
            for c in range(1024, HALF, 1024):
                nc.sync.dma_start(atb[:, c:c + 1024], at[:, c:c + 1024])
            colD = big.tile([128, M], f16, name="colD")

            def fold_tree(eng, src3d, nchunk, f_lo, F, tag):
                """Fold src3d [128, nchunk, CW] by halves down to FW, last
                level written into F[:, f_lo:f_lo+nchunk, :]."""
                cur, w = src3d, CW // 2
                while w >= FW:
                    if w == FW:
                        out = F[:, f_lo:f_lo + nchunk, :]
                    else:
                        out = scr.tile([128, nchunk, w], f16,
                                       tag=f"{tag}{w}", name=f"{tag}{w}")[:]
                    eng.tensor_tensor(out, cur[:, :, :w], cur[:, :, w:], op=mx)
                    cur, w = out, w // 2

            for t in range(NT):
                s16 = rows.tile([128, M], f16, tag="s16", name="s16")
                F = scr.tile([128, NCH, FW], f16, tag="F", name="F")
                for c in range(8):
                    pt = ps.tile([128, 1024], f32, tag="pt", name="pt")
                    for j in range(2):
                        mlo = c * 1024 + j * 512
                        nc.tensor.matmul(pt[:, j * 512:(j + 1) * 512],
                                         atb[:, t * 128:(t + 1) * 128],
                                         btb[:, mlo:mlo + 512],
                                         start=True, stop=True)
                    if c == 0:
                        # group 0 cast on DVE (ACT is the binding engine);
                        # phase 2 casts chunks 0-1 on DVE too, so both
                        # phases share one value lattice per chunk
                        nc.vector.tensor_copy(s16[:, 0:1024], pt[:])
                    else:
                        nc.scalar.copy(s16[:, c * 1024:(c + 1) * 1024], pt[:])

                # ---- DVE: fold tree over all 16 chunks (fp16)
                fold_tree(nc.vector,
                          s16[:].rearrange("p (c w) -> p c w", w=CW),
                          NCH, 0, F, "x")

                # ---- DVE: colD (full width, fp16 2x)
                if t == 0:
                    nc.vector.tensor_copy(colD[:], s16[:])
                else:
                    nc.vector.tensor_tensor(colD[:], colD[:], s16[:], op=mx)

                # stream this tile's floors out; host finishes the 16->1 max
                W = NCH * FW
                nc.sync.dma_start(cm_o[:, t * W:(t + 1) * W],
                                  F[:].rearrange("p c w -> p (c w)"))

            for c in range(0, M, 2048):
                nc.sync.dma_start(cd_o[:, c:c + 2048], colD[:, c:c + 2048])
    nc.compile()
    return nc


def _build2():
    f16, f32, u32 = mybir.dt.float16, mybir.dt.float32, mybir.dt.uint32
    nc = bacc.Bacc("TRN2", target_bir_lowering=False, debug=False,
                   num_devices=NCORES)
    at2 = nc.dram_tensor("at2", [D, NCH * PAD], f16, kind="ExternalInput").ap()
    bt = nc.dram_tensor("bt", [D, M], f16, kind="ExternalInput").ap()
    sg = nc.dram_tensor("sg", [128, NST], f16, kind="ExternalInput").ap()
    idx_o = nc.dram_tensor("idx", [128, NST * 8], u32, kind="ExternalOutput").ap()
    with tile.TileContext(nc) as tc:
        with tc.tile_pool(name="big", bufs=1) as big, \
             tc.tile_pool(name="work", bufs=6) as work, \
             tc.tile_pool(name="ps", bufs=4, space="PSUM") as ps:
            a2b = big.tile([128, NCH * PAD], f16, name="a2b")
            btb = big.tile([128, M], f16, name="btb")
            sgb = big.tile([128, NST], f16, name="sgb")
            # order matters: group 0's operands first, few serial issues
            nc.sync.dma_start(btb[:, 0:1024], bt[:, 0:1024])
            nc.sync.dma_start(a2b[:, 0:2 * PAD], at2[:, 0:2 * PAD])
            nc.sync.dma_start(sgb[:], sg[:])
            nc.sync.dma_start(a2b[:, 2 * PAD:8 * PAD], at2[:, 2 * PAD:8 * PAD])
            nc.sync.dma_start(btb[:, 1024:3072], bt[:, 1024:3072])
            nc.sync.dma_start(a2b[:, 8 * PAD:], at2[:, 8 * PAD:])
            for c in range(3072, M, 2048):
                hi = min(c + 2048, M)
                nc.sync.dma_start(btb[:, c:hi], bt[:, c:hi])
            idx8 = big.tile([128, NST * 8], u32, name="idx8")
            # broadcast all subtile scores to 8 copies each, once
            scb = big.tile([128, NST, 8], f16, name="scb")
            nc.vector.tensor_copy(
                scb[:], sgb[:].rearrange("p (c o) -> p c o", o=1)
                .to_broadcast((128, NST, 8)))
            KP = PAD // 128
            for p in range(NST // 2):
                st0 = 2 * p
                pt = ps.tile([128, 2 * CW], f32, tag="pt", name="pt")
                for k in range(2):
                    st = st0 + k
                    g = st // KP
                    nc.tensor.matmul(pt[:, k * CW:(k + 1) * CW],
                                     a2b[:, st * 128:(st + 1) * 128],
                                     btb[:, g * CW:(g + 1) * CW],
                                     start=True, stop=True)
                ch = work.tile([128, 2 * CW], f16, tag="ch", name="ch")
                if st0 < 6:
                    # subtiles 0-5 are chunk-groups 0-1 == phase-1 group 0:
                    # cast on DVE to match phase 1's rounding engine
                    nc.vector.tensor_copy(ch[:], pt[:])
                else:
                    nc.scalar.copy(ch[:], pt[:])
                for k in range(2):
                    st = st0 + k
                    nc.vector.max_index(idx8[:, st * 8:(st + 1) * 8],
                                        scb[:, st, :],
                                        ch[:, k * CW:(k + 1) * CW])
            nc.sync.dma_start(idx_o[:], idx8[:])
    nc.compile()
    return nc


_cached = None


def _make_exec(nc):
    import jax
    from jax.sharding import Mesh, PartitionSpec
    from jax.experimental.shard_map import shard_map
    from concourse import bass2jax
    from concourse.bass2jax import _bass_exec_p

    partition_name = nc.partition_id_tensor.name if nc.partition_id_tensor else None
    in_names, out_names, out_avals, out_shapes = [], [], [], []
    for alloc in nc.m.functions[0].allocations:
        if not isinstance(alloc, mybir.MemoryLocationSet):
            continue
        name = alloc.memorylocations[0].name
        if alloc.kind == "ExternalInput":
            if name != partition_name:
                in_names.append(name)
        elif alloc.kind == "ExternalOutput":
            shape = tuple(alloc.tensor_shape)
            dtype = mybir.dt.np(alloc.dtype)
            out_names.append(name)
            out_shapes.append((shape, dtype))
            out_avals.append(jax.core.ShapedArray(shape, dtype))
    n_params = len(in_names)
    n_outs = len(out_names)
    all_in_names = in_names + out_names
    if partition_name is not None:
        all_in_names = all_in_names + [partition_name]

    def _body(*args):
        operands = list(args)
        if partition_name is not None:
            operands.append(bass2jax.partition_id_tensor())
        outs = _bass_exec_p.bind(
            *operands, out_avals=tuple(out_avals), in_names=tuple(all_in_names),
            out_names=tuple(out_names), lowering_input_output_aliases=(),
            sim_require_finite=True, sim_require_nnan=True, nc=nc)
        return tuple(outs)

    devices = jax.devices()[:NCORES]
    mesh = Mesh(np.asarray(devices), ("core",))
    in_specs = (PartitionSpec("core"),) * (n_params + n_outs)
    out_specs = (PartitionSpec("core"),) * n_outs
    fn = jax.jit(shard_map(_body, mesh=mesh, in_specs=in_specs,
                           out_specs=out_specs, check_rep=False),
                 keep_unused=True)
    return {"fn": fn, "in_names": in_names, "out_names": out_names,
            "out_shapes": out_shapes, "nc": nc}


def _run(ex, ins):
    """ins: dict name -> [NCORES, *shape]; returns dict name -> [NCORES, *shape]."""
    concat_in = [np.ascontiguousarray(ins[n].reshape(-1, *ins[n].shape[2:]))
                 for n in ex["in_names"]]
    concat_zeros = [np.zeros((NCORES * s[0], *s[1:]), dt)
                    for (s, dt) in ex["out_shapes"]]
    out_arrs = ex["fn"](*concat_in, *concat_zeros)
    return {name: np.asarray(out_arrs[i]).reshape(NCORES, *ex["out_shapes"][i][0])
            for i, name in enumerate(ex["out_names"])}


def kernel(desc0, desc1):
    global _cached
    desc0 = np.asarray(desc0, dtype=np.float32)
    desc1 = np.asarray(desc1, dtype=np.float32)
    assert desc0.shape == (B, N, D) and desc1.shape == (B, M, D)

    if _cached is None:
        _cached = (_make_exec(_build1()), _make_exec(_build2()))
    ex1, ex2 = _cached

    a16 = desc0.astype(np.float16)
    b16 = desc1.astype(np.float16)
    a_slab = np.stack([a16[b, h * HALF:(h + 1) * HALF]
                       for b in range(B) for h in range(2)])      # [8,4096,128]
    bt_all = np.stack([b16[b].transpose(1, 0)
                       for b in range(B) for h in range(2)])      # [8,128,8192]
    at_all = np.ascontiguousarray(a_slab.transpose(0, 2, 1))      # [8,128,4096]

    r1 = _run(ex1, {"at": at_all, "bt": bt_all})

    # host glue: finish floor max, colmax assembly, chunk-argmax, grouping
    cm = r1["cm"].reshape(NCORES, 128, NT, NCH, FW).max(axis=4) \
                 .transpose(0, 2, 1, 3).reshape(NCORES, HALF, NCH)
    score0_c = cm.max(axis=2)                                     # [8,4096] f16
    cstar_c = cm.argmax(axis=2)                                   # [8,4096]

    colmax = r1["cd"].reshape(B, 2 * 128, M).max(axis=1)          # [B,M] f16

    at2 = np.zeros((NCORES, D, NCH * PAD), np.float16)
    sgv = np.full((NCORES, 128, NST), np.inf, np.float16)
    slot_of_row = np.full((NCORES, HALF), -1, np.int64)
    overflow = []                                                 # (core, row)
    for core in range(NCORES):
        for g in range(NCH):
            rows = np.nonzero(cstar_c[core] == g)[0]
            if len(rows) > PAD:
                overflow.extend((core, r) for r in rows[PAD:])
                rows = rows[:PAD]
            slots = g * PAD + np.arange(len(rows))
            slot_of_row[core, rows] = slots
            at2[core][:, slots] = a_slab[core][rows].T
            sgv[core][slots % 128, slots // 128] = score0_c[core][rows]

    r2 = _run(ex2, {"at2": at2, "bt": bt_all, "sg": sgv})
    within = r2["idx"][:, :, ::8]                                 # [8,128,NST]

    match01 = np.empty((B, N), dtype=np.int32)
    score0 = np.empty((B, N), dtype=np.float32)
    valid = np.empty((B, N), dtype=bool)

    for core in range(NCORES):
        b, h = divmod(core, 2)
        s = score0_c[core].astype(np.float32)
        sl = slot_of_row[core]
        m = cstar_c[core] * CW + \
            within[core][sl % 128, sl // 128].astype(np.int64)
        sel = slice(h * HALF, (h + 1) * HALF)
        score0[b, sel] = s
        match01[b, sel] = m.astype(np.int32)
        valid[b, sel] = (s > 0.1) & (score0_c[core] == colmax[b][m])

    for core, row in overflow:                                    # ~never taken
        b, h = divmod(core, 2)
        simrow = (a_slab[core][row].astype(np.float32)
                  @ b16[b].astype(np.float32).T).astype(np.float16)
        n = h * HALF + row
        match01[b, n] = int(simrow.argmax())
        score0[b, n] = float(simrow.max())
        valid[b, n] = (score0[b, n] > 0.1) & \
                      (simrow.max() == colmax[b][match01[b, n]])

    return match01, score0, valid
